# revision 11
# baseline (speedup 1.0000x reference)
"""Trainium2 Bass kernel for skipgram-style edge loss (embedding_lookup).

reference:
    u = emb[pos[:,0]]; v = emb[pos[:,1]]
    nu = emb[neg[...,0]]; nv = emb[neg[...,1]]
    loss = softplus(-<u,v>) + sum_k softplus(<nu_k,nv_k>)      # [E]

Strategy: replicate the 256MB table into each core's DRAM, split the 50k
edge batch 8 ways.  Each core performs row gathers via SWDGE indirect DMA
(one 512B descriptor per embedding row).  The second gather of each tile
multiplies element-wise into the first gather's SBUF tile via the SDMA
CCE datapath (compute_op=mult), so DVE only does the segmented reduces.
softplus is computed in product form: per edge
    loss = ln( prod_j (1 + exp(s_j * dot_j)) ),  s = (-1, +1 ... +1)
so Exp is the only per-tile ACT function (its table loads once) and a
single Ln runs at the end.

Task layout per core: edge e_local = (t*128 + p)*M + i maps to device
tile t, partition p, inner slot i; task j (0=pos, 1..5=neg) is the OUTER
slot dim (slot = j*M + i), so the pos/neg sign split is two contiguous
column ranges and folds into Exp's scale argument.
"""

import numpy as np

import concourse.bacc as bacc
import concourse.bass as bass
import concourse.mybir as mybir
from concourse.tile import TileContext
from concourse.bass_utils import run_bass_kernel_spmd

# Problem sizes (hardcoded per contract)
V = 500_000
D = 128
E = 50_000
K = 5

NCORES = 8
P = 128
J = K + 1                      # dot products per edge (1 pos + K neg)
EPC = E // NCORES              # 6250 edges per core
M = 7                          # edges per partition per tile
NT = -(-EPC // (P * M))        # 7 tiles per core
EPAD = NT * P * M              # 6272 padded edges per core
KSLOT = M * J                  # 42 dot slots per partition per tile

# NOTE: walrus rejects compute_op=mult on DMACopy (NCC_IBIR077), so the
# elementwise multiply must run on DVE; gathers are chunked so the DVE work
# pipelines against the SWDGE queue at ~1MB granularity.
USE_CCE_MULT = False

LAST_RESULTS = None            # BassKernelResults of the most recent run


def build_program(v=V, d=D, nt=NT, m=M, j=J, cce_mult=None):
    if cce_mult is None:
        cce_mult = USE_CCE_MULT
    kslot = m * j
    nc = bacc.Bacc(trn_type="TRN2")
    emb = nc.dram_tensor("embeddings", [v, d], mybir.dt.float32,
                         kind="ExternalInput")
    # [:, :nt*kslot] = left rows, [:, nt*kslot:] = right rows
    idx = nc.dram_tensor("idx", [P, 2 * nt * kslot], mybir.dt.int32,
                         kind="ExternalInput")
    loss = nc.dram_tensor("loss", [P, nt * m], mybir.dt.float32,
                          kind="ExternalOutput")

    with TileContext(nc) as tc:
        with (
            tc.tile_pool(name="io", bufs=1) as io_pool,
            tc.tile_pool(name="emb", bufs=6) as emb_pool,
            tc.tile_pool(name="small", bufs=3) as small_pool,
        ):
            idx_sb = io_pool.tile([P, 2 * nt * kslot], mybir.dt.int32)
            loss_sb = io_pool.tile([P, nt * m], mybir.dt.float32)
            nc.sync.dma_start(idx_sb[:], idx[:])

            def idxl(t):
                return idx_sb[:, t * kslot:(t + 1) * kslot]

            def idxr(t):
                return idx_sb[:, (nt + t) * kslot:(nt + t + 1) * kslot]

            # chunking: split each tile's kslot columns into j-aligned chunks
            # so gather->mul->reduce pipelines at sub-tile granularity and the
            # final tile's critical-path tail is one chunk, not a whole tile
            nchunks = 3 if kslot % 3 == 0 else 1
            csl = kslot // nchunks

            for t in range(nt):
                dots = small_pool.tile([P, kslot], mybir.dt.float32, tag="dots")
                for c in range(nchunks):
                    lo = c * csl
                    el = emb_pool.tile([P, csl * d], mybir.dt.bfloat16,
                                       tag="el")
                    er = emb_pool.tile([P, csl * d], mybir.dt.bfloat16,
                                       tag="er")
                    nc.gpsimd.indirect_dma_start(
                        out=el[:], out_offset=None, in_=emb[:],
                        in_offset=bass.IndirectOffsetOnAxis(
                            ap=idxl(t)[:, lo:lo + csl], axis=0))
                    nc.gpsimd.indirect_dma_start(
                        out=er[:], out_offset=None, in_=emb[:],
                        in_offset=bass.IndirectOffsetOnAxis(
                            ap=idxr(t)[:, lo:lo + csl], axis=0))
                    nc.vector.tensor_mul(el[:], el[:], er[:])
                    nc.vector.reduce_sum(
                        dots[:, lo:lo + csl],
                        el[:].rearrange("p (k d) -> p k d", d=d),
                        axis=mybir.AxisListType.X)

                # exp(s_j * dot): j=0 slots get scale -1 (positive edges)
                ex = small_pool.tile([P, kslot], mybir.dt.float32, tag="ex")
                nc.scalar.activation(ex[:, :m], dots[:, :m],
                                     mybir.ActivationFunctionType.Exp,
                                     scale=-1.0)
                nc.scalar.activation(ex[:, m:], dots[:, m:],
                                     mybir.ActivationFunctionType.Exp,
                                     scale=1.0)
                nc.vector.tensor_scalar_add(ex[:], ex[:], 1.0)
                # per-edge product over the J=6 tasks (j is the outer slot
                # dim): tree-multiply 6 -> 3 -> (2,1) -> 1 on cheap slices
                assert j == 6
                b = small_pool.tile([P, 3 * m], mybir.dt.float32, tag="b")
                c = small_pool.tile([P, m], mybir.dt.float32, tag="c")
                nc.vector.tensor_mul(b[:], ex[:, :3 * m], ex[:, 3 * m:])
                nc.vector.tensor_mul(c[:], b[:, :m], b[:, m:2 * m])
                nc.vector.tensor_mul(loss_sb[:, t * m:(t + 1) * m],
                                     c[:], b[:, 2 * m:])

            # loss = ln(prod), once over the whole [P, nt*m] result
            nc.scalar.activation(loss_sb[:], loss_sb[:],
                                 mybir.ActivationFunctionType.Ln)
            nc.sync.dma_start(loss[:], loss_sb[:])
    nc.finalize()
    return nc


def _pack_indices(pos_edges, neg_edges, core):
    """Build the [P, 2*NT*KSLOT] int32 row-index array for one core."""
    lo = core * EPC
    hi = lo + EPC
    tl = np.zeros((EPAD, J), np.int32)
    tr = np.zeros((EPAD, J), np.int32)
    tl[:EPC, 0] = pos_edges[lo:hi, 0]
    tl[:EPC, 1:] = neg_edges[lo:hi, :, 0]
    tr[:EPC, 0] = pos_edges[lo:hi, 1]
    tr[:EPC, 1:] = neg_edges[lo:hi, :, 1]
    # [EPAD, J] -> [NT, P, M, J] -> [P, NT, J, M] -> [P, NT*KSLOT]
    il = tl.reshape(NT, P, M, J).transpose(1, 0, 3, 2).reshape(P, NT * KSLOT)
    ir = tr.reshape(NT, P, M, J).transpose(1, 0, 3, 2).reshape(P, NT * KSLOT)
    return np.ascontiguousarray(np.concatenate([il, ir], axis=1))


_PROGRAM = None


def kernel(embeddings, pos_edges, neg_edges):
    global _PROGRAM, LAST_RESULTS
    embeddings = np.ascontiguousarray(np.asarray(embeddings, dtype=np.float32))
    pos_edges = np.asarray(pos_edges).astype(np.int32)
    neg_edges = np.asarray(neg_edges).astype(np.int32)

    if _PROGRAM is None:
        _PROGRAM = build_program()
    nc = _PROGRAM

    in_maps = [
        {"embeddings": embeddings,
         "idx": _pack_indices(pos_edges, neg_edges, c)}
        for c in range(NCORES)
    ]

    res = run_bass_kernel_spmd(nc, in_maps, core_ids=list(range(NCORES)))
    LAST_RESULTS = res

    out = np.empty(E, np.float32)
    for c in range(NCORES):
        dev = np.asarray(res.results[c]["loss"], np.float32)  # [P, NT*M]
        ordered = dev.reshape(P, NT, M).transpose(1, 0, 2).reshape(EPAD)
        out[c * EPC:(c + 1) * EPC] = ordered[:EPC]
    return out


# revision 13
# speedup vs baseline: 1.0025x; 1.0025x over previous
"""Trainium2 Bass kernel for skipgram-style edge loss (embedding_lookup).

reference:
    u = emb[pos[:,0]]; v = emb[pos[:,1]]
    nu = emb[neg[...,0]]; nv = emb[neg[...,1]]
    loss = softplus(-<u,v>) + sum_k softplus(<nu_k,nv_k>)      # [E]

Strategy: replicate the 256MB table into each core's DRAM, split the 50k
edge batch 8 ways.  Each core performs row gathers via SWDGE indirect DMA
(one 512B f32 descriptor per embedding row; bf16 cast-on-gather measured
SLOWER because 256B SBUF writes drop the SDMA engines below line rate).
DVE does elementwise mul + segmented reduce per ~1MB chunk so it pipelines
against the SWDGE queue; ACT applies softplus with the pos-edge sign flip
folded into the activation scale.

Task layout per core: edge e_local = (t*128 + p)*M + i maps to device
tile t, partition p, inner slot i; task j (0=pos, 1..5=neg) is the OUTER
slot dim (slot = j*M + i), so the pos/neg sign split is two contiguous
column ranges.
"""

import numpy as np

import concourse.bacc as bacc
import concourse.bass as bass
import concourse.mybir as mybir
from concourse.tile import TileContext
from concourse.bass_utils import run_bass_kernel_spmd

# Problem sizes (hardcoded per contract)
V = 500_000
D = 128
E = 50_000
K = 5

NCORES = 8
P = 128
J = K + 1                      # dot products per edge (1 pos + K neg)
EPC = E // NCORES              # 6250 edges per core
M = 7                          # edges per partition per tile
NT = -(-EPC // (P * M))        # 7 tiles per core
EPAD = NT * P * M              # 6272 padded edges per core
KSLOT = M * J                  # 42 dot slots per partition per tile

LAST_RESULTS = None            # BassKernelResults of the most recent run


def build_program(v=V, d=D, nt=NT, m=M, j=J, native_softplus=False):
    """native_softplus=True is unavailable: walrus has no ACT table entry for
    Softplus on this build ("no activation table contains Some(Softplus)").
    The default path computes softplus via exp + product tree + one final ln:
    sum_j ln(1+e^x_j) = ln prod_j (1+e^x_j)."""
    kslot = m * j
    nc = bacc.Bacc(trn_type="TRN2")
    emb = nc.dram_tensor("embeddings", [v, d], mybir.dt.float32,
                         kind="ExternalInput")
    # [:, :nt*kslot] = left rows, [:, nt*kslot:] = right rows
    idx = nc.dram_tensor("idx", [P, 2 * nt * kslot], mybir.dt.int32,
                         kind="ExternalInput")
    loss = nc.dram_tensor("loss", [P, nt * m], mybir.dt.float32,
                          kind="ExternalOutput")

    with TileContext(nc) as tc:
        with (
            tc.tile_pool(name="io", bufs=1) as io_pool,
            tc.tile_pool(name="idxp", bufs=4) as idx_pool,
            tc.tile_pool(name="emb", bufs=6) as emb_pool,
            tc.tile_pool(name="small", bufs=3) as small_pool,
        ):
            loss_sb = io_pool.tile([P, nt * m], mybir.dt.float32)

            # per-tile idx tiles: the first gather only waits on one tiny DMA
            idxl_sb = []
            idxr_sb = []
            for t in range(nt):
                tl = idx_pool.tile([P, kslot], mybir.dt.int32, tag=f"il{t}")
                tr = idx_pool.tile([P, kslot], mybir.dt.int32, tag=f"ir{t}")
                nc.sync.dma_start(tl[:], idx[:, t * kslot:(t + 1) * kslot])
                nc.sync.dma_start(
                    tr[:], idx[:, (nt + t) * kslot:(nt + t + 1) * kslot])
                idxl_sb.append(tl)
                idxr_sb.append(tr)

            # chunking: split each tile's kslot columns so gather->mul->reduce
            # pipelines at sub-tile granularity and the final tile's
            # critical-path tail is one chunk, not a whole tile
            nchunks = 3 if kslot % 3 == 0 else 1
            csl = kslot // nchunks

            for t in range(nt):
                dots = small_pool.tile([P, kslot], mybir.dt.float32, tag="dots")
                for c in range(nchunks):
                    lo = c * csl
                    el = emb_pool.tile([P, csl * d], mybir.dt.float32,
                                       tag="el")
                    er = emb_pool.tile([P, csl * d], mybir.dt.float32,
                                       tag="er")
                    nc.gpsimd.indirect_dma_start(
                        out=el[:], out_offset=None, in_=emb[:],
                        in_offset=bass.IndirectOffsetOnAxis(
                            ap=idxl_sb[t][:, lo:lo + csl], axis=0))
                    nc.gpsimd.indirect_dma_start(
                        out=er[:], out_offset=None, in_=emb[:],
                        in_offset=bass.IndirectOffsetOnAxis(
                            ap=idxr_sb[t][:, lo:lo + csl], axis=0))
                    nc.vector.tensor_mul(el[:], el[:], er[:])
                    nc.vector.reduce_sum(
                        dots[:, lo:lo + csl],
                        el[:].rearrange("p (k d) -> p k d", d=d),
                        axis=mybir.AxisListType.X)

                if native_softplus:
                    # softplus(s_j * dot): j=0 slots (positive edges) scale -1
                    sp = small_pool.tile([P, kslot], mybir.dt.float32,
                                         tag="sp")
                    nc.scalar.activation(
                        sp[:, :m], dots[:, :m],
                        mybir.ActivationFunctionType.Softplus, scale=-1.0)
                    nc.scalar.activation(
                        sp[:, m:], dots[:, m:],
                        mybir.ActivationFunctionType.Softplus, scale=1.0)
                    # loss = sum over the J tasks of each edge (stride-m cols)
                    nc.vector.reduce_sum(
                        loss_sb[:, t * m:(t + 1) * m],
                        sp[:].rearrange("p (j i) -> p i j", i=m),
                        axis=mybir.AxisListType.X)
                else:
                    # ln(prod_j (1 + exp(s_j dot_j))) via exp + product tree
                    ex = small_pool.tile([P, kslot], mybir.dt.float32,
                                         tag="ex")
                    nc.scalar.activation(ex[:, :m], dots[:, :m],
                                         mybir.ActivationFunctionType.Exp,
                                         scale=-1.0)
                    nc.scalar.activation(ex[:, m:], dots[:, m:],
                                         mybir.ActivationFunctionType.Exp,
                                         scale=1.0)
                    nc.vector.tensor_scalar_add(ex[:], ex[:], 1.0)
                    assert j == 6
                    b = small_pool.tile([P, 3 * m], mybir.dt.float32, tag="b")
                    cc = small_pool.tile([P, m], mybir.dt.float32, tag="c")
                    nc.vector.tensor_mul(b[:], ex[:, :3 * m], ex[:, 3 * m:])
                    nc.vector.tensor_mul(cc[:], b[:, :m], b[:, m:2 * m])
                    nc.vector.tensor_mul(loss_sb[:, t * m:(t + 1) * m],
                                         cc[:], b[:, 2 * m:])

            if not native_softplus:
                nc.scalar.activation(loss_sb[:], loss_sb[:],
                                     mybir.ActivationFunctionType.Ln)
            nc.sync.dma_start(loss[:], loss_sb[:])
    nc.finalize()
    return nc


def _pack_indices(pos_edges, neg_edges, core):
    """Build the [P, 2*NT*KSLOT] int32 row-index array for one core."""
    lo = core * EPC
    hi = lo + EPC
    tl = np.zeros((EPAD, J), np.int32)
    tr = np.zeros((EPAD, J), np.int32)
    tl[:EPC, 0] = pos_edges[lo:hi, 0]
    tl[:EPC, 1:] = neg_edges[lo:hi, :, 0]
    tr[:EPC, 0] = pos_edges[lo:hi, 1]
    tr[:EPC, 1:] = neg_edges[lo:hi, :, 1]
    # [EPAD, J] -> [NT, P, M, J] -> [P, NT, J, M] -> [P, NT*KSLOT]
    il = tl.reshape(NT, P, M, J).transpose(1, 0, 3, 2).reshape(P, NT * KSLOT)
    ir = tr.reshape(NT, P, M, J).transpose(1, 0, 3, 2).reshape(P, NT * KSLOT)
    return np.ascontiguousarray(np.concatenate([il, ir], axis=1))


_PROGRAM = None


def kernel(embeddings, pos_edges, neg_edges):
    global _PROGRAM, LAST_RESULTS
    embeddings = np.ascontiguousarray(np.asarray(embeddings, dtype=np.float32))
    pos_edges = np.asarray(pos_edges).astype(np.int32)
    neg_edges = np.asarray(neg_edges).astype(np.int32)

    if _PROGRAM is None:
        _PROGRAM = build_program()
    nc = _PROGRAM

    in_maps = [
        {"embeddings": embeddings,
         "idx": _pack_indices(pos_edges, neg_edges, c)}
        for c in range(NCORES)
    ]

    res = run_bass_kernel_spmd(nc, in_maps, core_ids=list(range(NCORES)))
    LAST_RESULTS = res

    out = np.empty(E, np.float32)
    for c in range(NCORES):
        dev = np.asarray(res.results[c]["loss"], np.float32)  # [P, NT*M]
        ordered = dev.reshape(P, NT, M).transpose(1, 0, 2).reshape(EPAD)
        out[c * EPC:(c + 1) * EPC] = ordered[:EPC]
    return out
